# revision 31
# baseline (speedup 1.0000x reference)
"""GPT2 attention (B=2, S=2048, E=1024, H=16) on 8 NeuronCores.

Sharding: tensor-parallel over heads - 2 heads per core. Each core computes
qkv^T for its heads, causal attention in transposed-score layout (k on
partitions, q on free dim), then a partial output projection over its 128
ctx dims. Host sums the 8 partials and adds b_proj.

v3 design notes (vs the 183us v2):
- ACT (scalar engine) decongested: it now runs ONLY the 80 exps. The proj
  psum evacuations, softmax-denominator copy and bf16 recip cast moved to
  DVE. The nk=16 blocks were ACT-bound end to end (16 exps + 4-8 evac
  copies ~= the whole block span); now they are bound by exp alone.
- Filler rebalance: every attention block hosts one qkv tile and/or one
  block's output projection (previously block (1,1) had no filler at all
  and stalled the PE long enough for the HAM clock to halve). proj filler
  units carry a gate_ki so they can't pop before the producing norm ran.
- Warmup no longer waits for make_identity: dummy matmuls read a junk
  tile, so the PE ramps the HAM clock from the very start of the body
  while the first DMAs are still in flight.
- DMA order: wqkv first half, first hsT quarters, then biases/masks, so
  the first QKV matmul has its operands ~3us earlier.
- Tail: keepalive matmuls interleaved into the final norm+proj chain keep
  the HAM clock at full rate while ACT/DVE finish up.
- PV matmuls carry the denominator in-psum (mask column in the vnat
  stationaries); scores run as row-tiled concurrent h0/h1 pairs.
"""
import os
from collections import deque

import numpy as np
import ml_dtypes

import concourse.bass as bass
import concourse.bacc as bacc
import concourse.tile as tile
from concourse import mybir
from concourse import masks
from concourse.bass_utils import run_bass_kernel_spmd

BF16 = ml_dtypes.bfloat16
B, S, E, H, D = 2, 2048, 1024, 16, 64
T = B * S                 # 4096 tokens
NCORE = 8
HPC = H // NCORE          # 2 heads per core
SCALE = D ** -0.5
F32 = mybir.dt.float32
BF = mybir.dt.bfloat16
EXP = mybir.ActivationFunctionType.Exp
CPY = mybir.ActivationFunctionType.Copy
N_WARMUP = 50

_built = {}


def _build():
    if "nc" in _built:
        return _built["nc"]
    nc = bacc.Bacc()
    hsTt = nc.declare_dram_parameter("hsTt", [8, 128, 8 * 512], BF,
                                     isOutput=False)
    wqkv = nc.declare_dram_parameter("wqkv", [128, 8, 3 * HPC * D], BF,
                                     isOutput=False)
    bqkv = nc.declare_dram_parameter("bqkv", [128, 3], F32, isOutput=False)
    wpT = nc.declare_dram_parameter("wpT", [HPC * D, E], BF, isOutput=False)
    maskb = nc.declare_dram_parameter("maskb", [128, 32], BF, isOutput=False)
    masks_ = nc.declare_dram_parameter("masks", [128, 32], F32, isOutput=False)
    out = nc.declare_dram_parameter("out", [T, E], BF, isOutput=True)


    with tile.TileContext(nc) as tc:
        with (
            tc.tile_pool(name="const", bufs=1) as constp,
            tc.tile_pool(name="big", bufs=1) as bigp,
            tc.tile_pool(name="hst", bufs=4) as hstp,
            tc.tile_pool(name="vtmp", bufs=2) as vtmpp,
            tc.tile_pool(name="expt", bufs=5) as exptp,
            tc.tile_pool(name="outp", bufs=4) as outp,
            tc.tile_pool(name="bcs", bufs=2) as bcsp,
            tc.tile_pool(name="recf", bufs=2) as recfp,
            tc.tile_pool(name="recb", bufs=2) as recbp,
            tc.tile_pool(name="ps_fill", bufs=2, space="PSUM") as fillp,
            tc.tile_pool(name="ps_sc", bufs=2, space="PSUM") as scp_pool,
            tc.tile_pool(name="ps_ctx", bufs=2, space="PSUM") as ctxp,
        ):
            # ---- constants ----
            wqkv_sb = constp.tile([128, 8, 384], BF)
            bq_sb = constp.tile([128, 3], F32)
            wpT_sb = constp.tile([128, E], BF)
            msk_sb = constp.tile([128, 32], F32)
            mskb_sb = constp.tile([128, 32], BF)
            ident = constp.tile([128, 128], BF)
            ones_sb = constp.tile([128, 64], BF)
            junk = constp.tile([128, 128], BF)

            qT = bigp.tile([128, T], BF)       # rows: h0 dims 0-63, h1 dims 64-127
            kT = bigp.tile([128, T], BF)
            ctxT = bigp.tile([128, T], BF)
            # vnatA: per 128-token chunk: [h0 dims 0:64, mask col 64]
            # vnatB: per 128-token chunk: [mask col 0, zeros 1:64, h1 dims 64:128]
            vnatA = bigp.tile([128, 32, 65], BF)
            vnatB = bigp.tile([128, 32, 128], BF)
            den2 = [bigp.tile([128, 512], F32, name=f"den{i}") for i in range(2)]

            # ---- hsT tile prefetch ----
            hst_tiles = {}

            def prefetch(n, quarters=False):
                ht = hstp.tile([128, 8, 512], BF, tag="ht", name=f"ht{n}")
                src_v = hsTt[n].rearrange("p (kc t) -> p kc t", kc=8)
                if quarters:
                    for q in range(4):
                        nc.sync.dma_start(out=ht[:, 2 * q:2 * q + 2, :],
                                          in_=src_v[:, 2 * q:2 * q + 2, :])
                else:
                    nc.sync.dma_start(out=ht[:, 0:4, :], in_=src_v[:, 0:4, :])
                    nc.sync.dma_start(out=ht[:, 4:8, :], in_=src_v[:, 4:8, :])
                hst_tiles[n] = ht

            # PE pre-warm on a junk tile (no dependency on DMA or identity
            # build): ramps the HAM clock gate to 8/8 while data streams in
            nc.vector.memset(junk, 1.0)
            wps = fillp.tile([128, 128], F32, tag="f", name="warm")

            def keep0(n):
                for _ in range(n):
                    nc.tensor.matmul(wps, lhsT=junk, rhs=junk,
                                     start=True, stop=True)

            keep0(N_WARMUP)

            # DMA order: k-major interleave of wqkv / hsT-tile-0 quarters so
            # the first QKV matmuls start as soon as ~350KB have landed and
            # then pace with the DMA stream
            ht0 = hstp.tile([128, 8, 512], BF, tag="ht", name="ht0")
            src0 = hsTt[0].rearrange("p (kc t) -> p kc t", kc=8)
            for q in range(4):
                nc.sync.dma_start(out=wqkv_sb[:, 2 * q:2 * q + 2, :],
                                  in_=wqkv[:, 2 * q:2 * q + 2, :])
                nc.sync.dma_start(out=ht0[:, 2 * q:2 * q + 2, :],
                                  in_=src0[:, 2 * q:2 * q + 2, :])
            hst_tiles[0] = ht0
            nc.sync.dma_start(out=bq_sb, in_=bqkv[:])
            nc.sync.dma_start(out=msk_sb, in_=masks_[:])
            nc.sync.dma_start(out=mskb_sb, in_=maskb[:])
            prefetch(1)
            nc.sync.dma_start(out=wpT_sb, in_=wpT[:])
            masks.make_identity(nc, ident[:])
            nc.vector.memset(ones_sb, 1.0)
            nc.vector.memset(den2[0], 1.0)
            nc.vector.memset(den2[1], 1.0)
            nc.gpsimd.memset(vnatB[:, :, 1:64], 0.0)
            nc.vector.tensor_copy(vnatA[:, :, 64:65], mskb_sb)
            nc.vector.tensor_copy(vnatB[:, :, 0:1], mskb_sb)

            # ---- qkv tile for 512 tokens: filler units ----
            vtmp_of = {}

            def qkv_units(n, paced=False, split_v=False):
                units = []
                pm = {}

                def mk_mm(m, klo, khi):
                    def u():
                        if klo == 0:
                            pm[m] = fillp.tile([128, 512], F32, tag="f",
                                               name=f"qkv{n}_{m}")
                        for k in range(klo, khi):
                            nc.tensor.matmul(
                                pm[m], lhsT=wqkv_sb[:, k, m * 128:(m + 1) * 128],
                                rhs=hst_tiles[n][:, k, :],
                                start=(k == 0), stop=(k == 7),
                            )
                    return u

                def mk_ev(m):
                    def u():
                        if m == 0:
                            nc.vector.tensor_scalar_add(
                                qT[:, n * 512:(n + 1) * 512], pm[0], bq_sb[:, 0:1])
                        elif m == 1:
                            nc.vector.tensor_scalar_add(
                                kT[:, n * 512:(n + 1) * 512], pm[1], bq_sb[:, 1:2])
                        else:
                            vt = vtmpp.tile([128, 512], BF, tag="vt",
                                            name=f"vt{n}")
                            nc.vector.tensor_scalar_add(vt, pm[2], bq_sb[:, 2:3])
                            vtmp_of[n] = vt
                    return u

                def mk_tr(t):
                    def u():
                        pst = fillp.tile([128, 128], BF, tag="f",
                                         name=f"tr{n}_{t}")
                        nc.tensor.transpose(
                            pst[:], vtmp_of[n][:, t * 128:(t + 1) * 128], ident[:])
                        tt4 = n * 4 + t
                        nc.vector.tensor_scalar_mul(
                            vnatA[:, tt4, 0:64], pst[:, 0:64],
                            msk_sb[:, tt4:tt4 + 1])
                        nc.vector.tensor_scalar_mul(
                            vnatB[:, tt4, 64:128], pst[:, 64:128],
                            msk_sb[:, tt4:tt4 + 1])
                    return u

                for m in range(3):
                    for klo in range(0, 8, 2):
                        units.append((0, mk_mm(m, klo, klo + 2)))
                        if paced and m == 0 and klo < 6:
                            # cover the DMA-arrival gaps of the k-major
                            # quarter stream with PE keepalives
                            units.append((0, lambda: keep0(6)))
                    units.append((0, mk_ev(m)))
                for t in range(4):
                    units.append((0, mk_tr(t)))
                if split_v:
                    # [qk-part, v-part]: the v/vnat units can run a block
                    # later than q/k (vnat is only read by the PVs)
                    return units[:10], units[10:]
                return units

            # ---- output projection for one qj block: filler units ----
            ot_of = {}

            def proj_units(b, qj, gate=0, act_evac=True):
                # act_evac=False keeps the n2=1 evac on DVE: used when the
                # hosting block is ACT-bound (nk=16), so ACT only runs exps
                units = []

                def mk_pj(t, n2):
                    tc_ = (4 * b + qj) * 4 + t

                    def u():
                        pp = fillp.tile([128, 512], F32, tag="f",
                                        name=f"pj{tc_}_{n2}")
                        nc.tensor.matmul(
                            pp, lhsT=ctxT[:, tc_ * 128:(tc_ + 1) * 128],
                            rhs=wpT_sb[:, n2 * 512:(n2 + 1) * 512],
                            start=True, stop=True,
                        )
                        if n2 == 0:
                            ot = outp.tile([128, 1024], BF, tag="ot",
                                           name=f"ot{tc_}")
                            ot_of[tc_] = ot
                            nc.vector.tensor_copy(ot[:, 0:512], pp)
                        else:
                            ot = ot_of.pop(tc_)
                            if act_evac:
                                nc.scalar.activation(out=ot[:, 512:1024],
                                                     in_=pp, func=CPY)
                            else:
                                nc.vector.tensor_copy(ot[:, 512:1024], pp)
                            nc.sync.dma_start(
                                out=out[tc_ * 128:(tc_ + 1) * 128, :],
                                in_=ot,
                            )
                    return u

                for t in range(4):
                    for n2 in range(2):
                        units.append((gate, mk_pj(t, n2)))
                return units

            # ---- score+exp chain for chunk ki of block (b, qj) ----
            # shared by the in-loop path and the cross-block lookahead
            exps_g = {}

            def emit_score_exp(b, qj, ki):
                scp = scp_pool.tile([128, 1024], F32, tag="sc",
                                    name=f"sc{b}{qj}_{ki}")
                qsl = slice(b * S + qj * 512, b * S + (qj + 1) * 512)
                ksl = slice(b * S + ki * 128, b * S + (ki + 1) * 128)
                nc.tensor.matmul(scp[:, 0:512], lhsT=kT[0:64, ksl],
                                 rhs=qT[0:64, qsl], start=True, stop=True)
                nc.tensor.matmul(scp[:, 512:1024], lhsT=kT[64:128, ksl],
                                 rhs=qT[64:128, qsl], start=True, stop=True)
                e = exptp.tile([128, 1024], BF, tag="e",
                               name=f"ex{b}{qj}_{ki}")
                nc.scalar.activation(out=e, in_=scp, func=EXP, scale=SCALE)
                d = ki - 4 * qj
                if d >= 0:   # diagonal: zero where k > q
                    for hh in range(2):
                        sl = e[:, hh * 512:(hh + 1) * 512]
                        nc.gpsimd.affine_select(
                            out=sl, in_=sl,
                            compare_op=mybir.AluOpType.is_ge, fill=0.0,
                            base=-(128 * d), channel_multiplier=-1,
                            pattern=[[1, 512]],
                        )
                exps_g[(b, qj, ki)] = e

            # ---- causal attention for one (b, qj) 512-query block ----
            # la: closure emitting the NEXT block's first score+exp chains
            # before this block's PV tail, so the next block's ACT work
            # starts ~2us earlier (the boundary was an ACT bubble).
            def attention(b, qj, fq, pending=None, la=None):
                nk = 4 * qj + 4
                ctxA = ctxp.tile([128, 512], F32, tag="ctx", name=f"cA{b}{qj}")
                ctxB = ctxp.tile([128, 512], F32, tag="ctx", name=f"cB{b}{qj}")
                qsl = slice(b * S + qj * 512, b * S + (qj + 1) * 512)

                def pop_filler(k, ki):
                    for _ in range(k):
                        if fq and fq[0][0] <= ki:
                            fq.popleft()[1]()

                def emit_pv(ki):
                    e = exps_g.pop((b, qj, ki))
                    kc = b * 16 + ki
                    nc.tensor.matmul(
                        ctxA[0:65, :], lhsT=vnatA[:, kc, :], rhs=e[:, 0:512],
                        start=(ki == 0), stop=(ki == nk - 1),
                    )
                    nc.tensor.matmul(
                        ctxB[:, :], lhsT=vnatB[:, kc, :], rhs=e[:, 512:1024],
                        start=(ki == 0), stop=(ki == nk - 1),
                    )

                for ki in range(nk):
                    if (b, qj, ki) not in exps_g:
                        emit_score_exp(b, qj, ki)
                    if ki == 1 and pending is not None:
                        pending()
                    slots_left = nk - ki + 1
                    pop_filler(min(4, -(-len(fq) // slots_left)), ki)
                    if ki >= 2:
                        emit_pv(ki - 2)
                # lookahead: next block's ki=0,1 score+exp before our tail
                if la is not None:
                    la()
                emit_pv(nk - 2)
                pop_filler(max(0, len(fq) - 4), nk)
                emit_pv(nk - 1)
                # drain most filler BEFORE the norm chain below so the next
                # block's qT/kT evacs aren't queued behind it on DVE (this
                # was a ~2us PE stall at every block boundary)
                pop_filler(max(0, len(fq) - 4), nk)

                # normalize part 1 (no PE ops): h0 denom = ctxA row 64,
                # h1 denom = ctxB row 0. ACT copies row 64 / DVE row 0 (rows
                # 64/0 of a memset-once tile), one base-0 approx-reciprocal
                # covers both rows, one ACT cast to bf16 for the broadcast.
                # split ACT/DVE: at block ends DVE is congested (filler
                # evacs, vnat muls) while ACT only has the lookahead exps
                den = den2[(4 * b + qj) % 2]
                nc.scalar.activation(out=den[64:65, :], in_=ctxA[64:65, :],
                                     func=CPY)
                nc.vector.tensor_copy(den[0:1, :], ctxB[0:1, :])
                recf = recfp.tile([128, 512], F32, tag="rf", name=f"rf{b}{qj}")
                recb = recbp.tile([128, 512], BF, tag="rb", name=f"rb{b}{qj}")
                nc.vector.reciprocal_approx_fast(recf[0:65, :], den[0:65, :])
                with nc.allow_low_precision(reason="bf16 recip, rel<2e-2 ok"):
                    nc.scalar.activation(out=recb[0:65, :], in_=recf[0:65, :],
                                         func=CPY)
                pop_filler(len(fq), nk)

                def norm_tail():
                    # part 2 (PE bcast + DVE muls) - deferred into the next
                    # attention block so the PE queue never stalls on recips
                    bps = scp_pool.tile([128, 512], F32, tag="sc",
                                        name=f"bp{b}{qj}")
                    nc.tensor.matmul(bps[0:64, :], lhsT=ones_sb[64:65, :],
                                     rhs=recb[64:65, :], start=True, stop=True)
                    nc.tensor.matmul(bps[64:128, :], lhsT=ones_sb[0:1, :],
                                     rhs=recb[0:1, :], start=True, stop=True)
                    bcs = bcsp.tile([128, 512], BF, tag="bc", name=f"bc{b}{qj}")
                    # split the broadcast-evac and the muls so the first
                    # 128-token chunk's ctxT (what the first proj matmul
                    # reads) is ready ~1.2us earlier
                    q0 = qsl.start
                    nc.vector.tensor_copy(bcs[:, 0:128], bps[:, 0:128])
                    nc.vector.tensor_mul(ctxT[0:64, q0:q0 + 128],
                                         ctxA[0:64, 0:128], bcs[0:64, 0:128])
                    nc.vector.tensor_mul(ctxT[64:128, q0:q0 + 128],
                                         ctxB[64:128, 0:128],
                                         bcs[64:128, 0:128])
                    nc.vector.tensor_copy(bcs[:, 128:512], bps[:, 128:512])
                    nc.vector.tensor_mul(ctxT[0:64, q0 + 128:q0 + 512],
                                         ctxA[0:64, 128:512],
                                         bcs[0:64, 128:512])
                    nc.vector.tensor_mul(ctxT[64:128, q0 + 128:q0 + 512],
                                         ctxB[64:128, 128:512],
                                         bcs[64:128, 128:512])
                return norm_tail

            # ---- main schedule ----
            for _, u in qkv_units(0, paced=True):
                u()
            # proj filler assignment per block index 0..7 (block = 4b+qj):
            # every block hosts the previous block's projection; the
            # ACT-bound nk=16 block 7 has PE slack for two blocks' proj.
            # gate=2 delays pops until the producing norm (run at ki==1 via
            # `pending`) is in the queue.
            proj_fill = {1: [((0, 0), 3, True)], 2: [((0, 1), 3, True)],
                         3: [((0, 2), 3, False)], 4: [((0, 3), 3, True)],
                         5: [((1, 0), 3, True)],
                         7: [((1, 1), 0, False), ((1, 2), 3, False)]}
            def mk_la(nb, nqj):
                def la():
                    emit_score_exp(nb, nqj, 0)
                    emit_score_exp(nb, nqj, 1)
                return la

            pending = None
            v7_units = None
            for b in range(B):
                for qj in range(4):
                    tt = 4 * b + qj
                    if tt + 2 <= 7:
                        prefetch(tt + 2)
                    fq = deque()
                    if tt == 6:
                        # tile 7's v/vnat units become block 7's PE filler
                        # (block 7 otherwise starves and HAM-throttles);
                        # its vnat isn't read before ki=12 there
                        qk7, v7_units = qkv_units(7, split_v=True)
                        fq.extend(qk7)
                    elif tt + 1 <= 7:
                        fq.extend(qkv_units(tt + 1))
                    if tt == 7:
                        # interleave tile-7 v/vnat units with proj(1,1) so
                        # consecutive fill-pool psum allocations belong to
                        # independent chains (halves the evac-wait stalls)
                        p11 = proj_units(1, 1, 0, False)
                        head, trs = v7_units[:5], v7_units[5:]
                        fq.extend(head)
                        for i in range(4):
                            fq.append(trs[i])
                            fq.append(p11[i])
                        fq.extend(p11[4:])
                        fq.extend(proj_units(1, 2, 2, False))
                    else:
                        for (pb, pq), gate, ae in proj_fill.get(tt, []):
                            fq.extend(proj_units(pb, pq, gate, ae))
                    la = (mk_la((tt + 1) // 4, (tt + 1) % 4)
                          if tt < 7 else None)
                    pending = attention(b, qj, fq, pending, la)

            # ---- tail: norm + proj of block (1,3) with PE keepalives so
            # the HAM clock stays at 8/8 through the serial ACT/DVE chain
            kps = scp_pool.tile([128, 512], F32, tag="sc", name="keep")

            def keep(n):
                for _ in range(n):
                    nc.tensor.matmul(kps[0:128, 0:128], lhsT=junk, rhs=junk,
                                     start=True, stop=True)

            keep(32)
            pending()
            keep(16)
            # final projection: alternate psum pools (fillp/ctxp) so the 8
            # matmuls pipeline across 4 live buffers instead of stalling on
            # the 2-buffer fill pool behind each chunk's evac
            for t in range(4):
                tc_ = 28 + t
                ot = outp.tile([128, 1024], BF, tag="ot", name=f"ot{tc_}")
                for n2 in range(2):
                    if n2 == 0:
                        pp = fillp.tile([128, 512], F32, tag="f",
                                        name=f"pj{tc_}_0")
                    else:
                        pp = ctxp.tile([128, 512], F32, tag="ctx",
                                       name=f"pj{tc_}_1")
                    nc.tensor.matmul(
                        pp, lhsT=ctxT[:, tc_ * 128:(tc_ + 1) * 128],
                        rhs=wpT_sb[:, n2 * 512:(n2 + 1) * 512],
                        start=True, stop=True,
                    )
                    if n2 == 0:
                        nc.vector.tensor_copy(ot[:, 0:512], pp)
                    else:
                        nc.scalar.activation(out=ot[:, 512:1024], in_=pp,
                                             func=CPY)
                        nc.sync.dma_start(
                            out=out[tc_ * 128:(tc_ + 1) * 128, :], in_=ot)
                keep(2)
            keep(8)
    nc.finalize()
    _built["nc"] = nc
    return nc


def kernel(hidden_states, attention_mask, W_attn, b_attn, W_proj, b_proj,
           _trace=False):
    hs = np.asarray(hidden_states, np.float32).reshape(T, E)
    # [tile, partition, kc, col] with 8KB contiguous per partition line
    hsTt = np.ascontiguousarray(
        hs.reshape(8, 512, 8, 128).transpose(0, 3, 2, 1).reshape(8, 128, 4096)
    ).astype(BF16)
    mask = np.asarray(attention_mask)
    mcol = (mask.reshape(B * S) != 0).astype(np.float32)        # [4096]
    mchunk = np.ascontiguousarray(mcol.reshape(32, 128).T)       # [128, 32]
    maskb = mchunk.astype(BF16)
    masks_ = mchunk.astype(np.float32)
    W_attn = np.asarray(W_attn, np.float32)
    W_proj = np.asarray(W_proj, np.float32)
    b_attn = np.asarray(b_attn, np.float32)

    in_maps = []
    for c in range(NCORE):
        rows = np.concatenate(
            [np.arange(sec * E + c * 128, sec * E + (c + 1) * 128)
             for sec in range(3)]
        )
        wq = np.ascontiguousarray(
            W_attn[rows].T.reshape(8, 128, 384).transpose(1, 0, 2)
        ).astype(BF16)                                               # [128,8,384]
        bq = np.ascontiguousarray(
            b_attn[rows].reshape(3, 128).T).astype(np.float32)             # [128,3] f32
        wp = np.ascontiguousarray(W_proj[:, c * 128:(c + 1) * 128].T).astype(BF16)
        in_maps.append(
            {"hsTt": hsTt, "wqkv": wq, "bqkv": bq, "wpT": wp,
             "maskb": maskb, "masks": masks_}
        )

    nc = _build()
    res = run_bass_kernel_spmd(nc, in_maps, list(range(NCORE)), trace=_trace)
    parts = np.stack([np.asarray(r["out"], np.float32) for r in res.results])
    outv = parts.sum(axis=0) + np.asarray(b_proj, np.float32)[None, :]
    out = outv.reshape(B, S, E).astype(np.float32)
    if _trace:
        return out, res
    return out


# revision 33
# speedup vs baseline: 1.1619x; 1.1619x over previous
"""GPT2 attention (B=2, S=2048, E=1024, H=16) on 8 NeuronCores.

Sharding: tensor-parallel over heads - 2 heads per core. Each core computes
qkv^T for its heads, causal attention in transposed-score layout (k on
partitions, q on free dim), then a partial output projection over its 128
ctx dims. Host sums the 8 partials and adds b_proj.

v3 design notes (vs the 183us v2):
- ACT (scalar engine) decongested: it now runs ONLY the 80 exps. The proj
  psum evacuations, softmax-denominator copy and bf16 recip cast moved to
  DVE. The nk=16 blocks were ACT-bound end to end (16 exps + 4-8 evac
  copies ~= the whole block span); now they are bound by exp alone.
- Filler rebalance: every attention block hosts one qkv tile and/or one
  block's output projection (previously block (1,1) had no filler at all
  and stalled the PE long enough for the HAM clock to halve). proj filler
  units carry a gate_ki so they can't pop before the producing norm ran.
- Warmup no longer waits for make_identity: dummy matmuls read a junk
  tile, so the PE ramps the HAM clock from the very start of the body
  while the first DMAs are still in flight.
- DMA order: wqkv first half, first hsT quarters, then biases/masks, so
  the first QKV matmul has its operands ~3us earlier.
- Tail: keepalive matmuls interleaved into the final norm+proj chain keep
  the HAM clock at full rate while ACT/DVE finish up.
- PV matmuls carry the denominator in-psum (mask column in the vnat
  stationaries); scores run as row-tiled concurrent h0/h1 pairs.
"""
import os
from collections import deque

import numpy as np
import ml_dtypes

import concourse.bass as bass
import concourse.bacc as bacc
import concourse.tile as tile
from concourse import mybir
from concourse import masks
from concourse.bass_utils import run_bass_kernel_spmd

BF16 = ml_dtypes.bfloat16
B, S, E, H, D = 2, 2048, 1024, 16, 64
T = B * S                 # 4096 tokens
NCORE = 8
HPC = H // NCORE          # 2 heads per core
SCALE = D ** -0.5
F32 = mybir.dt.float32
BF = mybir.dt.bfloat16
EXP = mybir.ActivationFunctionType.Exp
CPY = mybir.ActivationFunctionType.Copy
N_WARMUP = 50

_built = {}


def _build():
    if "nc" in _built:
        return _built["nc"]
    nc = bacc.Bacc()
    hsTt = nc.declare_dram_parameter("hsTt", [8, 128, 8 * 512], BF,
                                     isOutput=False)
    wqkv = nc.declare_dram_parameter("wqkv", [128, 8, 3 * HPC * D], BF,
                                     isOutput=False)
    bqkv = nc.declare_dram_parameter("bqkv", [128, 3], F32, isOutput=False)
    wpT = nc.declare_dram_parameter("wpT", [HPC * D, E], BF, isOutput=False)
    maskb = nc.declare_dram_parameter("maskb", [128, 32], BF, isOutput=False)
    masks_ = nc.declare_dram_parameter("masks", [128, 32], F32, isOutput=False)
    out = nc.declare_dram_parameter("out", [T, E], BF, isOutput=True)


    with tile.TileContext(nc) as tc:
        with (
            tc.tile_pool(name="const", bufs=1) as constp,
            tc.tile_pool(name="big", bufs=1) as bigp,
            tc.tile_pool(name="hst", bufs=4) as hstp,
            tc.tile_pool(name="vtmp", bufs=2) as vtmpp,
            tc.tile_pool(name="expt", bufs=5) as exptp,
            tc.tile_pool(name="outp", bufs=4) as outp,
            tc.tile_pool(name="bcs", bufs=2) as bcsp,
            tc.tile_pool(name="recf", bufs=2) as recfp,
            tc.tile_pool(name="recb", bufs=2) as recbp,
            tc.tile_pool(name="ps_fill", bufs=2, space="PSUM") as fillp,
            tc.tile_pool(name="ps_sc", bufs=2, space="PSUM") as scp_pool,
            tc.tile_pool(name="ps_ctx", bufs=2, space="PSUM") as ctxp,
        ):
            # ---- constants ----
            wqkv_sb = constp.tile([128, 8, 384], BF)
            bq_sb = constp.tile([128, 3], F32)
            wpT_sb = constp.tile([128, E], BF)
            msk_sb = constp.tile([128, 32], F32)
            mskb_sb = constp.tile([128, 32], BF)
            ident = constp.tile([128, 128], BF)
            ones_sb = constp.tile([128, 64], BF)
            junk = constp.tile([128, 128], BF)

            qT = bigp.tile([128, T], BF)       # rows: h0 dims 0-63, h1 dims 64-127
            kT = bigp.tile([128, T], BF)
            ctxT = bigp.tile([128, T], BF)
            # vnatA: per 128-token chunk: [h0 dims 0:64, mask col 64]
            # vnatB: per 128-token chunk: [mask col 0, zeros 1:64, h1 dims 64:128]
            vnatA = bigp.tile([128, 32, 65], BF)
            vnatB = bigp.tile([128, 32, 128], BF)
            den2 = [bigp.tile([128, 512], F32, name=f"den{i}") for i in range(2)]

            # ---- hsT tile prefetch ----
            hst_tiles = {}

            def prefetch(n, quarters=False):
                ht = hstp.tile([128, 8, 512], BF, tag="ht", name=f"ht{n}")
                src_v = hsTt[n].rearrange("p (kc t) -> p kc t", kc=8)
                if quarters:
                    for q in range(4):
                        nc.sync.dma_start(out=ht[:, 2 * q:2 * q + 2, :],
                                          in_=src_v[:, 2 * q:2 * q + 2, :])
                else:
                    nc.sync.dma_start(out=ht[:, 0:4, :], in_=src_v[:, 0:4, :])
                    nc.sync.dma_start(out=ht[:, 4:8, :], in_=src_v[:, 4:8, :])
                hst_tiles[n] = ht

            # PE pre-warm on a junk tile (no dependency on DMA or identity
            # build): ramps the HAM clock gate to 8/8 while data streams in
            nc.vector.memset(junk, 1.0)
            wps = fillp.tile([128, 128], F32, tag="f", name="warm")

            def keep0(n):
                for _ in range(n):
                    nc.tensor.matmul(wps, lhsT=junk, rhs=junk,
                                     start=True, stop=True)

            keep0(N_WARMUP)

            # DMA order: k-major interleave of wqkv / hsT-tile-0 quarters so
            # the first QKV matmuls start as soon as ~350KB have landed and
            # then pace with the DMA stream
            ht0 = hstp.tile([128, 8, 512], BF, tag="ht", name="ht0")
            src0 = hsTt[0].rearrange("p (kc t) -> p kc t", kc=8)
            for q in range(4):
                nc.sync.dma_start(out=wqkv_sb[:, 2 * q:2 * q + 2, :],
                                  in_=wqkv[:, 2 * q:2 * q + 2, :])
                nc.sync.dma_start(out=ht0[:, 2 * q:2 * q + 2, :],
                                  in_=src0[:, 2 * q:2 * q + 2, :])
            hst_tiles[0] = ht0
            nc.sync.dma_start(out=bq_sb, in_=bqkv[:])
            nc.sync.dma_start(out=msk_sb, in_=masks_[:])
            nc.sync.dma_start(out=mskb_sb, in_=maskb[:])
            prefetch(1)
            nc.sync.dma_start(out=wpT_sb, in_=wpT[:])
            masks.make_identity(nc, ident[:])
            nc.vector.memset(ones_sb, 1.0)
            nc.vector.memset(den2[0], 1.0)
            nc.vector.memset(den2[1], 1.0)
            nc.gpsimd.memset(vnatB[:, :, 1:64], 0.0)
            nc.vector.tensor_copy(vnatA[:, :, 64:65], mskb_sb)
            nc.vector.tensor_copy(vnatB[:, :, 0:1], mskb_sb)

            # ---- qkv tile for 512 tokens: filler units ----
            vtmp_of = {}

            def qkv_units(n, paced=False, split_v=False):
                units = []
                pm = {}

                def mk_mm(m, klo, khi):
                    def u():
                        if klo == 0:
                            pm[m] = fillp.tile([128, 512], F32, tag="f",
                                               name=f"qkv{n}_{m}")
                        for k in range(klo, khi):
                            nc.tensor.matmul(
                                pm[m], lhsT=wqkv_sb[:, k, m * 128:(m + 1) * 128],
                                rhs=hst_tiles[n][:, k, :],
                                start=(k == 0), stop=(k == 7),
                            )
                    return u

                def mk_ev(m):
                    def u():
                        if m == 0:
                            nc.vector.tensor_scalar_add(
                                qT[:, n * 512:(n + 1) * 512], pm[0], bq_sb[:, 0:1])
                        elif m == 1:
                            nc.vector.tensor_scalar_add(
                                kT[:, n * 512:(n + 1) * 512], pm[1], bq_sb[:, 1:2])
                        else:
                            vt = vtmpp.tile([128, 512], BF, tag="vt",
                                            name=f"vt{n}")
                            nc.vector.tensor_scalar_add(vt, pm[2], bq_sb[:, 2:3])
                            vtmp_of[n] = vt
                    return u

                def mk_tr(t):
                    def u():
                        pst = fillp.tile([128, 128], BF, tag="f",
                                         name=f"tr{n}_{t}")
                        nc.tensor.transpose(
                            pst[:], vtmp_of[n][:, t * 128:(t + 1) * 128], ident[:])
                        tt4 = n * 4 + t
                        nc.vector.tensor_scalar_mul(
                            vnatA[:, tt4, 0:64], pst[:, 0:64],
                            msk_sb[:, tt4:tt4 + 1])
                        nc.vector.tensor_scalar_mul(
                            vnatB[:, tt4, 64:128], pst[:, 64:128],
                            msk_sb[:, tt4:tt4 + 1])
                    return u

                for m in range(3):
                    for klo in range(0, 8, 2):
                        units.append((0, mk_mm(m, klo, klo + 2)))
                        if paced and m == 0 and klo < 6:
                            # cover the DMA-arrival gaps of the k-major
                            # quarter stream with PE keepalives
                            units.append((0, lambda: keep0(6)))
                    units.append((0, mk_ev(m)))
                for t in range(4):
                    units.append((0, mk_tr(t)))
                if split_v:
                    # [qk-part, v-part]: the v/vnat units can run a block
                    # later than q/k (vnat is only read by the PVs)
                    return units[:10], units[10:]
                return units

            # ---- output projection for one qj block: filler units ----
            ot_of = {}

            def proj_units(b, qj, gate=0, act_evac=True):
                # act_evac=False keeps the n2=1 evac on DVE: used when the
                # hosting block is ACT-bound (nk=16), so ACT only runs exps
                units = []

                def mk_pj(t, n2):
                    tc_ = (4 * b + qj) * 4 + t

                    def u():
                        pp = fillp.tile([128, 512], F32, tag="f",
                                        name=f"pj{tc_}_{n2}")
                        nc.tensor.matmul(
                            pp, lhsT=ctxT[:, tc_ * 128:(tc_ + 1) * 128],
                            rhs=wpT_sb[:, n2 * 512:(n2 + 1) * 512],
                            start=True, stop=True,
                        )
                        if n2 == 0:
                            ot = outp.tile([128, 1024], BF, tag="ot",
                                           name=f"ot{tc_}")
                            ot_of[tc_] = ot
                            nc.vector.tensor_copy(ot[:, 0:512], pp)
                        else:
                            ot = ot_of.pop(tc_)
                            if act_evac:
                                nc.scalar.activation(out=ot[:, 512:1024],
                                                     in_=pp, func=CPY)
                            else:
                                nc.vector.tensor_copy(ot[:, 512:1024], pp)
                            nc.sync.dma_start(
                                out=out[tc_ * 128:(tc_ + 1) * 128, :],
                                in_=ot,
                            )
                    return u

                for t in range(4):
                    for n2 in range(2):
                        units.append((gate, mk_pj(t, n2)))
                return units

            # ---- score+exp chain for chunk ki of block (b, qj) ----
            # shared by the in-loop path and the cross-block lookahead
            exps_g = {}

            def emit_score_exp(b, qj, ki):
                scp = scp_pool.tile([128, 1024], F32, tag="sc",
                                    name=f"sc{b}{qj}_{ki}")
                qsl = slice(b * S + qj * 512, b * S + (qj + 1) * 512)
                ksl = slice(b * S + ki * 128, b * S + (ki + 1) * 128)
                nc.tensor.matmul(scp[:, 0:512], lhsT=kT[0:64, ksl],
                                 rhs=qT[0:64, qsl], start=True, stop=True)
                nc.tensor.matmul(scp[:, 512:1024], lhsT=kT[64:128, ksl],
                                 rhs=qT[64:128, qsl], start=True, stop=True)
                e = exptp.tile([128, 1024], BF, tag="e",
                               name=f"ex{b}{qj}_{ki}")
                nc.scalar.activation(out=e, in_=scp, func=EXP, scale=SCALE)
                d = ki - 4 * qj
                if d >= 0:   # diagonal: zero where k > q
                    for hh in range(2):
                        sl = e[:, hh * 512:(hh + 1) * 512]
                        nc.gpsimd.affine_select(
                            out=sl, in_=sl,
                            compare_op=mybir.AluOpType.is_ge, fill=0.0,
                            base=-(128 * d), channel_multiplier=-1,
                            pattern=[[1, 512]],
                        )
                exps_g[(b, qj, ki)] = e

            # ---- causal attention for one (b, qj) 512-query block ----
            # la: closure emitting the NEXT block's first score+exp chains
            # before this block's PV tail, so the next block's ACT work
            # starts ~2us earlier (the boundary was an ACT bubble).
            def attention(b, qj, fq, pending=None, la=None):
                nk = 4 * qj + 4
                ctxA = ctxp.tile([128, 512], F32, tag="ctx", name=f"cA{b}{qj}")
                ctxB = ctxp.tile([128, 512], F32, tag="ctx", name=f"cB{b}{qj}")
                qsl = slice(b * S + qj * 512, b * S + (qj + 1) * 512)

                def pop_filler(k, ki):
                    # skip-scan: take the first ELIGIBLE unit so a gated
                    # proj head never blocks independent qkv/vnat units
                    for _ in range(k):
                        for idx in range(len(fq)):
                            if fq[idx][0] <= ki:
                                u = fq[idx][1]
                                del fq[idx]
                                u()
                                break
                        else:
                            break

                def emit_pv(ki):
                    e = exps_g.pop((b, qj, ki))
                    kc = b * 16 + ki
                    nc.tensor.matmul(
                        ctxA[0:65, :], lhsT=vnatA[:, kc, :], rhs=e[:, 0:512],
                        start=(ki == 0), stop=(ki == nk - 1),
                    )
                    nc.tensor.matmul(
                        ctxB[:, :], lhsT=vnatB[:, kc, :], rhs=e[:, 512:1024],
                        start=(ki == 0), stop=(ki == nk - 1),
                    )

                for ki in range(nk):
                    if (b, qj, ki) not in exps_g:
                        emit_score_exp(b, qj, ki)
                    if ki == 1 and pending is not None:
                        pending()
                    slots_left = nk - ki + 1
                    pop_filler(min(4, -(-len(fq) // slots_left)), ki)
                    if ki >= 2:
                        emit_pv(ki - 2)
                # lookahead: next block's ki=0,1 score+exp before our tail
                if la is not None:
                    la()
                emit_pv(nk - 2)
                pop_filler(max(0, len(fq) - 4), nk)
                emit_pv(nk - 1)
                # drain most filler BEFORE the norm chain below so the next
                # block's qT/kT evacs aren't queued behind it on DVE (this
                # was a ~2us PE stall at every block boundary)
                pop_filler(max(0, len(fq) - 4), nk)

                # normalize part 1 (no PE ops): h0 denom = ctxA row 64,
                # h1 denom = ctxB row 0. ACT copies row 64 / DVE row 0 (rows
                # 64/0 of a memset-once tile), one base-0 approx-reciprocal
                # covers both rows, one ACT cast to bf16 for the broadcast.
                # split ACT/DVE: at block ends DVE is congested (filler
                # evacs, vnat muls) while ACT only has the lookahead exps
                den = den2[(4 * b + qj) % 2]
                nc.scalar.activation(out=den[64:65, :], in_=ctxA[64:65, :],
                                     func=CPY)
                nc.vector.tensor_copy(den[0:1, :], ctxB[0:1, :])
                recf = recfp.tile([128, 512], F32, tag="rf", name=f"rf{b}{qj}")
                recb = recbp.tile([128, 512], BF, tag="rb", name=f"rb{b}{qj}")
                nc.vector.reciprocal_approx_fast(recf[0:65, :], den[0:65, :])
                with nc.allow_low_precision(reason="bf16 recip, rel<2e-2 ok"):
                    nc.scalar.activation(out=recb[0:65, :], in_=recf[0:65, :],
                                         func=CPY)
                pop_filler(len(fq), nk)

                def norm_tail():
                    # part 2 (PE bcast + DVE muls) - deferred into the next
                    # attention block so the PE queue never stalls on recips
                    bps = scp_pool.tile([128, 512], F32, tag="sc",
                                        name=f"bp{b}{qj}")
                    nc.tensor.matmul(bps[0:64, :], lhsT=ones_sb[64:65, :],
                                     rhs=recb[64:65, :], start=True, stop=True)
                    nc.tensor.matmul(bps[64:128, :], lhsT=ones_sb[0:1, :],
                                     rhs=recb[0:1, :], start=True, stop=True)
                    bcs = bcsp.tile([128, 512], BF, tag="bc", name=f"bc{b}{qj}")
                    # split the broadcast-evac and the muls so the first
                    # 128-token chunk's ctxT (what the first proj matmul
                    # reads) is ready ~1.2us earlier
                    q0 = qsl.start
                    nc.vector.tensor_copy(bcs[:, 0:128], bps[:, 0:128])
                    nc.vector.tensor_mul(ctxT[0:64, q0:q0 + 128],
                                         ctxA[0:64, 0:128], bcs[0:64, 0:128])
                    nc.vector.tensor_mul(ctxT[64:128, q0:q0 + 128],
                                         ctxB[64:128, 0:128],
                                         bcs[64:128, 0:128])
                    nc.vector.tensor_copy(bcs[:, 128:512], bps[:, 128:512])
                    nc.vector.tensor_mul(ctxT[0:64, q0 + 128:q0 + 512],
                                         ctxA[0:64, 128:512],
                                         bcs[0:64, 128:512])
                    nc.vector.tensor_mul(ctxT[64:128, q0 + 128:q0 + 512],
                                         ctxB[64:128, 128:512],
                                         bcs[64:128, 128:512])
                return norm_tail

            # ---- main schedule ----
            for _, u in qkv_units(0, paced=True):
                u()
            # proj filler assignment per block index 0..7 (block = 4b+qj):
            # every block hosts the previous block's projection; the
            # ACT-bound nk=16 block 7 has PE slack for two blocks' proj.
            # gate=2 delays pops until the producing norm (run at ki==1 via
            # `pending`) is in the queue.
            proj_fill = {1: [((0, 0), 3, True)], 2: [((0, 1), 3, True)],
                         3: [((0, 2), 3, False)], 4: [((0, 3), 3, True)],
                         5: [((1, 0), 3, True)],
                         7: [((1, 1), 0, False), ((1, 2), 3, False)]}
            def mk_la(nb, nqj):
                def la():
                    emit_score_exp(nb, nqj, 0)
                    emit_score_exp(nb, nqj, 1)
                return la

            pending = None
            v7_units = None
            for b in range(B):
                for qj in range(4):
                    tt = 4 * b + qj
                    if tt + 2 <= 7:
                        prefetch(tt + 2)
                    # interleave the qkv v-part / proj units so consecutive
                    # fill-pool psum allocations belong to independent
                    # chains (halves the evac-wait stalls on the 2-buffer
                    # ring); skip-scan pops keep gated units from blocking
                    fq = deque()
                    pj = []
                    for (pb, pq), gate, ae in proj_fill.get(tt, []):
                        pj.extend(proj_units(pb, pq, gate, ae))
                    if tt == 6:
                        # tile 7's v/vnat units become block 7's PE filler
                        # (block 7 otherwise starves and HAM-throttles);
                        # its vnat isn't read before ki=12 there
                        qk7, v7_units = qkv_units(7, split_v=True)
                        fq.extend(qk7)
                    elif tt + 1 <= 7:
                        qk, vp = qkv_units(tt + 1, split_v=True)
                        fq.extend(qk)
                        while vp or pj:
                            if vp:
                                fq.append(vp.pop(0))
                            if pj:
                                fq.append(pj.pop(0))
                        pj = []
                    if tt == 7:
                        pj = proj_units(1, 1, 0, False)
                        head, trs = v7_units[:5], v7_units[5:]
                        fq.extend(head)
                        for i in range(4):
                            fq.append(trs[i])
                            fq.append(pj[i])
                        fq.extend(pj[4:])
                        fq.extend(proj_units(1, 2, 2, False))
                        pj = []
                    fq.extend(pj)
                    la = (mk_la((tt + 1) // 4, (tt + 1) % 4)
                          if tt < 7 else None)
                    pending = attention(b, qj, fq, pending, la)

            # ---- tail: norm + proj of block (1,3) with PE keepalives so
            # the HAM clock stays at 8/8 through the serial ACT/DVE chain
            kps = scp_pool.tile([128, 512], F32, tag="sc", name="keep")

            def keep(n):
                for _ in range(n):
                    nc.tensor.matmul(kps[0:128, 0:128], lhsT=junk, rhs=junk,
                                     start=True, stop=True)

            keep(32)
            pending()
            keep(16)
            # final projection: alternate psum pools (fillp/ctxp) so the 8
            # matmuls pipeline across 4 live buffers instead of stalling on
            # the 2-buffer fill pool behind each chunk's evac
            for t in range(4):
                tc_ = 28 + t
                ot = outp.tile([128, 1024], BF, tag="ot", name=f"ot{tc_}")
                for n2 in range(2):
                    if n2 == 0:
                        pp = fillp.tile([128, 512], F32, tag="f",
                                        name=f"pj{tc_}_0")
                    else:
                        pp = ctxp.tile([128, 512], F32, tag="ctx",
                                       name=f"pj{tc_}_1")
                    nc.tensor.matmul(
                        pp, lhsT=ctxT[:, tc_ * 128:(tc_ + 1) * 128],
                        rhs=wpT_sb[:, n2 * 512:(n2 + 1) * 512],
                        start=True, stop=True,
                    )
                    if n2 == 0:
                        nc.vector.tensor_copy(ot[:, 0:512], pp)
                    else:
                        nc.scalar.activation(out=ot[:, 512:1024], in_=pp,
                                             func=CPY)
                        nc.sync.dma_start(
                            out=out[tc_ * 128:(tc_ + 1) * 128, :], in_=ot)
                keep(2)
            keep(8)
    nc.finalize()
    _built["nc"] = nc
    return nc


def kernel(hidden_states, attention_mask, W_attn, b_attn, W_proj, b_proj,
           _trace=False):
    hs = np.asarray(hidden_states, np.float32).reshape(T, E)
    # [tile, partition, kc, col] with 8KB contiguous per partition line
    hsTt = np.ascontiguousarray(
        hs.reshape(8, 512, 8, 128).transpose(0, 3, 2, 1).reshape(8, 128, 4096)
    ).astype(BF16)
    mask = np.asarray(attention_mask)
    mcol = (mask.reshape(B * S) != 0).astype(np.float32)        # [4096]
    mchunk = np.ascontiguousarray(mcol.reshape(32, 128).T)       # [128, 32]
    maskb = mchunk.astype(BF16)
    masks_ = mchunk.astype(np.float32)
    W_attn = np.asarray(W_attn, np.float32)
    W_proj = np.asarray(W_proj, np.float32)
    b_attn = np.asarray(b_attn, np.float32)

    in_maps = []
    for c in range(NCORE):
        rows = np.concatenate(
            [np.arange(sec * E + c * 128, sec * E + (c + 1) * 128)
             for sec in range(3)]
        )
        wq = np.ascontiguousarray(
            W_attn[rows].T.reshape(8, 128, 384).transpose(1, 0, 2)
        ).astype(BF16)                                               # [128,8,384]
        bq = np.ascontiguousarray(
            b_attn[rows].reshape(3, 128).T).astype(np.float32)             # [128,3] f32
        wp = np.ascontiguousarray(W_proj[:, c * 128:(c + 1) * 128].T).astype(BF16)
        in_maps.append(
            {"hsTt": hsTt, "wqkv": wq, "bqkv": bq, "wpT": wp,
             "maskb": maskb, "masks": masks_}
        )

    nc = _build()
    res = run_bass_kernel_spmd(nc, in_maps, list(range(NCORE)), trace=_trace)
    parts = np.stack([np.asarray(r["out"], np.float32) for r in res.results])
    outv = parts.sum(axis=0) + np.asarray(b_proj, np.float32)[None, :]
    out = outv.reshape(B, S, E).astype(np.float32)
    if _trace:
        return out, res
    return out


# revision 37
# speedup vs baseline: 1.1635x; 1.0014x over previous
"""GPT2 attention (B=2, S=2048, E=1024, H=16) on 8 NeuronCores.

Sharding: tensor-parallel over heads - 2 heads per core. Each core computes
qkv^T for its heads, causal attention in transposed-score layout (k on
partitions, q on free dim), then a partial output projection over its 128
ctx dims. Host sums the 8 partials and adds b_proj.

v3 design notes (vs the 183us v2):
- ACT (scalar engine) decongested: it now runs ONLY the 80 exps. The proj
  psum evacuations, softmax-denominator copy and bf16 recip cast moved to
  DVE. The nk=16 blocks were ACT-bound end to end (16 exps + 4-8 evac
  copies ~= the whole block span); now they are bound by exp alone.
- Filler rebalance: every attention block hosts one qkv tile and/or one
  block's output projection (previously block (1,1) had no filler at all
  and stalled the PE long enough for the HAM clock to halve). proj filler
  units carry a gate_ki so they can't pop before the producing norm ran.
- Warmup no longer waits for make_identity: dummy matmuls read a junk
  tile, so the PE ramps the HAM clock from the very start of the body
  while the first DMAs are still in flight.
- DMA order: wqkv first half, first hsT quarters, then biases/masks, so
  the first QKV matmul has its operands ~3us earlier.
- Tail: keepalive matmuls interleaved into the final norm+proj chain keep
  the HAM clock at full rate while ACT/DVE finish up.
- PV matmuls carry the denominator in-psum (mask column in the vnat
  stationaries); scores run as row-tiled concurrent h0/h1 pairs.
"""
import os
from collections import deque

import numpy as np
import ml_dtypes

import concourse.bass as bass
import concourse.bacc as bacc
import concourse.tile as tile
from concourse import mybir
from concourse import masks
from concourse.bass_utils import run_bass_kernel_spmd

BF16 = ml_dtypes.bfloat16
B, S, E, H, D = 2, 2048, 1024, 16, 64
T = B * S                 # 4096 tokens
NCORE = 8
HPC = H // NCORE          # 2 heads per core
SCALE = D ** -0.5
F32 = mybir.dt.float32
BF = mybir.dt.bfloat16
EXP = mybir.ActivationFunctionType.Exp
CPY = mybir.ActivationFunctionType.Copy
N_WARMUP = 45

_built = {}


def _build():
    if "nc" in _built:
        return _built["nc"]
    nc = bacc.Bacc()
    hsTt = nc.declare_dram_parameter("hsTt", [8, 128, 8 * 512], BF,
                                     isOutput=False)
    wqkv = nc.declare_dram_parameter("wqkv", [128, 8, 3 * HPC * D], BF,
                                     isOutput=False)
    bqkv = nc.declare_dram_parameter("bqkv", [128, 3], F32, isOutput=False)
    wpT = nc.declare_dram_parameter("wpT", [HPC * D, E], BF, isOutput=False)
    maskb = nc.declare_dram_parameter("maskb", [128, 32], BF, isOutput=False)
    masks_ = nc.declare_dram_parameter("masks", [128, 32], F32, isOutput=False)
    out = nc.declare_dram_parameter("out", [T, E], BF, isOutput=True)


    with tile.TileContext(nc) as tc:
        with (
            tc.tile_pool(name="const", bufs=1) as constp,
            tc.tile_pool(name="big", bufs=1) as bigp,
            tc.tile_pool(name="hst", bufs=4) as hstp,
            tc.tile_pool(name="vtmp", bufs=2) as vtmpp,
            tc.tile_pool(name="expt", bufs=5) as exptp,
            tc.tile_pool(name="outp", bufs=4) as outp,
            tc.tile_pool(name="bcs", bufs=2) as bcsp,
            tc.tile_pool(name="recf", bufs=2) as recfp,
            tc.tile_pool(name="recb", bufs=2) as recbp,
            tc.tile_pool(name="ps_fill", bufs=2, space="PSUM") as fillp,
            tc.tile_pool(name="ps_sc", bufs=2, space="PSUM") as scp_pool,
            tc.tile_pool(name="ps_ctx", bufs=2, space="PSUM") as ctxp,
        ):
            # ---- constants ----
            wqkv_sb = constp.tile([128, 8, 384], BF)
            bq_sb = constp.tile([128, 3], F32)
            wpT_sb = constp.tile([128, E], BF)
            msk_sb = constp.tile([128, 32], F32)
            mskb_sb = constp.tile([128, 32], BF)
            ident = constp.tile([128, 128], BF)
            ones_sb = constp.tile([128, 64], BF)
            junk = constp.tile([128, 128], BF)

            qT = bigp.tile([128, T], BF)       # rows: h0 dims 0-63, h1 dims 64-127
            kT = bigp.tile([128, T], BF)
            ctxT = bigp.tile([128, T], BF)
            # vnatA: per 128-token chunk: [h0 dims 0:64, mask col 64]
            # vnatB: per 128-token chunk: [mask col 0, zeros 1:64, h1 dims 64:128]
            vnatA = bigp.tile([128, 32, 65], BF)
            vnatB = bigp.tile([128, 32, 128], BF)
            den2 = [bigp.tile([128, 512], F32, name=f"den{i}") for i in range(2)]

            # ---- hsT tile prefetch ----
            hst_tiles = {}

            def prefetch(n, quarters=False):
                ht = hstp.tile([128, 8, 512], BF, tag="ht", name=f"ht{n}")
                src_v = hsTt[n].rearrange("p (kc t) -> p kc t", kc=8)
                if quarters:
                    for q in range(4):
                        nc.sync.dma_start(out=ht[:, 2 * q:2 * q + 2, :],
                                          in_=src_v[:, 2 * q:2 * q + 2, :])
                else:
                    nc.sync.dma_start(out=ht[:, 0:4, :], in_=src_v[:, 0:4, :])
                    nc.sync.dma_start(out=ht[:, 4:8, :], in_=src_v[:, 4:8, :])
                hst_tiles[n] = ht

            # PE pre-warm on a junk tile (no dependency on DMA or identity
            # build): ramps the HAM clock gate to 8/8 while data streams in
            nc.vector.memset(junk, 1.0)
            wps = fillp.tile([128, 128], F32, tag="f", name="warm")

            def keep0(n):
                for _ in range(n):
                    nc.tensor.matmul(wps, lhsT=junk, rhs=junk,
                                     start=True, stop=True)

            keep0(N_WARMUP)

            # DMA order: k-major interleave of wqkv / hsT-tile-0 quarters so
            # the first QKV matmuls start as soon as ~350KB have landed and
            # then pace with the DMA stream
            ht0 = hstp.tile([128, 8, 512], BF, tag="ht", name="ht0")
            src0 = hsTt[0].rearrange("p (kc t) -> p kc t", kc=8)
            for q in range(4):
                nc.sync.dma_start(out=wqkv_sb[:, 2 * q:2 * q + 2, :],
                                  in_=wqkv[:, 2 * q:2 * q + 2, :])
                nc.sync.dma_start(out=ht0[:, 2 * q:2 * q + 2, :],
                                  in_=src0[:, 2 * q:2 * q + 2, :])
            hst_tiles[0] = ht0
            nc.sync.dma_start(out=bq_sb, in_=bqkv[:])
            nc.sync.dma_start(out=msk_sb, in_=masks_[:])
            nc.sync.dma_start(out=mskb_sb, in_=maskb[:])
            prefetch(1)
            nc.sync.dma_start(out=wpT_sb, in_=wpT[:])
            masks.make_identity(nc, ident[:])
            nc.vector.memset(ones_sb, 1.0)
            nc.vector.memset(den2[0], 1.0)
            nc.vector.memset(den2[1], 1.0)
            nc.gpsimd.memset(vnatB[:, :, 1:64], 0.0)
            nc.vector.tensor_copy(vnatA[:, :, 64:65], mskb_sb)
            nc.vector.tensor_copy(vnatB[:, :, 0:1], mskb_sb)

            # ---- qkv tile for 512 tokens: filler units ----
            vtmp_of = {}

            def qkv_units(n, paced=False, split_v=False):
                units = []
                pm = {}

                def mk_mm(m, klo, khi):
                    def u():
                        if klo == 0:
                            pm[m] = fillp.tile([128, 512], F32, tag="f",
                                               name=f"qkv{n}_{m}")
                        for k in range(klo, khi):
                            nc.tensor.matmul(
                                pm[m], lhsT=wqkv_sb[:, k, m * 128:(m + 1) * 128],
                                rhs=hst_tiles[n][:, k, :],
                                start=(k == 0), stop=(k == 7),
                            )
                    return u

                def mk_ev(m):
                    def u():
                        if m == 0:
                            nc.vector.tensor_scalar_add(
                                qT[:, n * 512:(n + 1) * 512], pm[0], bq_sb[:, 0:1])
                        elif m == 1:
                            nc.vector.tensor_scalar_add(
                                kT[:, n * 512:(n + 1) * 512], pm[1], bq_sb[:, 1:2])
                        else:
                            vt = vtmpp.tile([128, 512], BF, tag="vt",
                                            name=f"vt{n}")
                            nc.vector.tensor_scalar_add(vt, pm[2], bq_sb[:, 2:3])
                            vtmp_of[n] = vt
                    return u

                def mk_tr(t):
                    def u():
                        pst = fillp.tile([128, 128], BF, tag="f",
                                         name=f"tr{n}_{t}")
                        nc.tensor.transpose(
                            pst[:], vtmp_of[n][:, t * 128:(t + 1) * 128], ident[:])
                        tt4 = n * 4 + t
                        nc.vector.tensor_scalar_mul(
                            vnatA[:, tt4, 0:64], pst[:, 0:64],
                            msk_sb[:, tt4:tt4 + 1])
                        nc.vector.tensor_scalar_mul(
                            vnatB[:, tt4, 64:128], pst[:, 64:128],
                            msk_sb[:, tt4:tt4 + 1])
                    return u

                for m in range(3):
                    for klo in range(0, 8, 2):
                        units.append((0, mk_mm(m, klo, klo + 2)))
                        if paced and m == 0 and klo < 6:
                            # cover the DMA-arrival gaps of the k-major
                            # quarter stream with PE keepalives
                            units.append((0, lambda: keep0(6)))
                    units.append((0, mk_ev(m)))
                for t in range(4):
                    units.append((0, mk_tr(t)))
                if split_v:
                    # [qk-part, v-part]: the v/vnat units can run a block
                    # later than q/k (vnat is only read by the PVs)
                    return units[:10], units[10:]
                return units

            # ---- output projection for one qj block: filler units ----
            ot_of = {}

            def proj_units(b, qj, gate=0, act_evac=True):
                # act_evac=False keeps the n2=1 evac on DVE: used when the
                # hosting block is ACT-bound (nk=16), so ACT only runs exps
                units = []

                def mk_pj(t, n2):
                    tc_ = (4 * b + qj) * 4 + t

                    def u():
                        pp = fillp.tile([128, 512], F32, tag="f",
                                        name=f"pj{tc_}_{n2}")
                        nc.tensor.matmul(
                            pp, lhsT=ctxT[:, tc_ * 128:(tc_ + 1) * 128],
                            rhs=wpT_sb[:, n2 * 512:(n2 + 1) * 512],
                            start=True, stop=True,
                        )
                        if n2 == 0:
                            ot = outp.tile([128, 1024], BF, tag="ot",
                                           name=f"ot{tc_}")
                            ot_of[tc_] = ot
                            nc.vector.tensor_copy(ot[:, 0:512], pp)
                        else:
                            ot = ot_of.pop(tc_)
                            if act_evac:
                                nc.scalar.activation(out=ot[:, 512:1024],
                                                     in_=pp, func=CPY)
                            else:
                                nc.vector.tensor_copy(ot[:, 512:1024], pp)
                            nc.sync.dma_start(
                                out=out[tc_ * 128:(tc_ + 1) * 128, :],
                                in_=ot,
                            )
                    return u

                for t in range(4):
                    for n2 in range(2):
                        units.append((gate, mk_pj(t, n2)))
                return units

            # ---- score+exp chain for chunk ki of block (b, qj) ----
            # shared by the in-loop path and the cross-block lookahead
            exps_g = {}

            def emit_score_exp(b, qj, ki):
                scp = scp_pool.tile([128, 1024], F32, tag="sc",
                                    name=f"sc{b}{qj}_{ki}")
                qsl = slice(b * S + qj * 512, b * S + (qj + 1) * 512)
                ksl = slice(b * S + ki * 128, b * S + (ki + 1) * 128)
                nc.tensor.matmul(scp[:, 0:512], lhsT=kT[0:64, ksl],
                                 rhs=qT[0:64, qsl], start=True, stop=True)
                nc.tensor.matmul(scp[:, 512:1024], lhsT=kT[64:128, ksl],
                                 rhs=qT[64:128, qsl], start=True, stop=True)
                e = exptp.tile([128, 1024], BF, tag="e",
                               name=f"ex{b}{qj}_{ki}")
                nc.scalar.activation(out=e, in_=scp, func=EXP, scale=SCALE)
                d = ki - 4 * qj
                if d >= 0:   # diagonal: zero where k > q
                    for hh in range(2):
                        sl = e[:, hh * 512:(hh + 1) * 512]
                        nc.gpsimd.affine_select(
                            out=sl, in_=sl,
                            compare_op=mybir.AluOpType.is_ge, fill=0.0,
                            base=-(128 * d), channel_multiplier=-1,
                            pattern=[[1, 512]],
                        )
                exps_g[(b, qj, ki)] = e

            # ---- causal attention for one (b, qj) 512-query block ----
            # la: closure emitting the NEXT block's first score+exp chains
            # before this block's PV tail, so the next block's ACT work
            # starts ~2us earlier (the boundary was an ACT bubble).
            def attention(b, qj, fq, pending=None, la=None):
                nk = 4 * qj + 4
                ctxA = ctxp.tile([128, 512], F32, tag="ctx", name=f"cA{b}{qj}")
                ctxB = ctxp.tile([128, 512], F32, tag="ctx", name=f"cB{b}{qj}")
                qsl = slice(b * S + qj * 512, b * S + (qj + 1) * 512)

                def pop_filler(k, ki):
                    for _ in range(k):
                        if fq and fq[0][0] <= ki:
                            fq.popleft()[1]()

                def emit_pv(ki):
                    e = exps_g.pop((b, qj, ki))
                    kc = b * 16 + ki
                    nc.tensor.matmul(
                        ctxA[0:65, :], lhsT=vnatA[:, kc, :], rhs=e[:, 0:512],
                        start=(ki == 0), stop=(ki == nk - 1),
                    )
                    nc.tensor.matmul(
                        ctxB[:, :], lhsT=vnatB[:, kc, :], rhs=e[:, 512:1024],
                        start=(ki == 0), stop=(ki == nk - 1),
                    )

                for ki in range(nk):
                    if (b, qj, ki) not in exps_g:
                        emit_score_exp(b, qj, ki)
                    if ki == 1 and pending is not None:
                        pending()
                    slots_left = nk - ki + 1
                    pop_filler(min(4, -(-len(fq) // slots_left)), ki)
                    if ki >= 2:
                        emit_pv(ki - 2)
                # lookahead: next block's ki=0,1 score+exp before our tail
                if la is not None:
                    la()
                emit_pv(nk - 2)
                pop_filler(max(0, len(fq) - 4), nk)
                emit_pv(nk - 1)
                # drain most filler BEFORE the norm chain below so the next
                # block's qT/kT evacs aren't queued behind it on DVE (this
                # was a ~2us PE stall at every block boundary)
                pop_filler(max(0, len(fq) - 4), nk)

                # normalize part 1 (no PE ops): h0 denom = ctxA row 64,
                # h1 denom = ctxB row 0. ACT copies row 64 / DVE row 0 (rows
                # 64/0 of a memset-once tile), one base-0 approx-reciprocal
                # covers both rows, one ACT cast to bf16 for the broadcast.
                # split ACT/DVE: at block ends DVE is congested (filler
                # evacs, vnat muls) while ACT only has the lookahead exps
                den = den2[(4 * b + qj) % 2]
                nc.scalar.activation(out=den[64:65, :], in_=ctxA[64:65, :],
                                     func=CPY)
                nc.vector.tensor_copy(den[0:1, :], ctxB[0:1, :])
                recf = recfp.tile([128, 512], F32, tag="rf", name=f"rf{b}{qj}")
                recb = recbp.tile([128, 512], BF, tag="rb", name=f"rb{b}{qj}")
                nc.vector.reciprocal_approx_fast(recf[0:65, :], den[0:65, :])
                with nc.allow_low_precision(reason="bf16 recip, rel<2e-2 ok"):
                    nc.scalar.activation(out=recb[0:65, :], in_=recf[0:65, :],
                                         func=CPY)
                pop_filler(len(fq), nk)

                def norm_tail():
                    # part 2 (PE bcast + DVE muls) - deferred into the next
                    # attention block so the PE queue never stalls on recips
                    bps = scp_pool.tile([128, 512], F32, tag="sc",
                                        name=f"bp{b}{qj}")
                    nc.tensor.matmul(bps[0:64, :], lhsT=ones_sb[64:65, :],
                                     rhs=recb[64:65, :], start=True, stop=True)
                    nc.tensor.matmul(bps[64:128, :], lhsT=ones_sb[0:1, :],
                                     rhs=recb[0:1, :], start=True, stop=True)
                    bcs = bcsp.tile([128, 512], BF, tag="bc", name=f"bc{b}{qj}")
                    # split the broadcast-evac and the muls so the first
                    # 128-token chunk's ctxT (what the first proj matmul
                    # reads) is ready ~1.2us earlier
                    q0 = qsl.start
                    nc.vector.tensor_copy(bcs[:, 0:128], bps[:, 0:128])
                    nc.vector.tensor_mul(ctxT[0:64, q0:q0 + 128],
                                         ctxA[0:64, 0:128], bcs[0:64, 0:128])
                    nc.vector.tensor_mul(ctxT[64:128, q0:q0 + 128],
                                         ctxB[64:128, 0:128],
                                         bcs[64:128, 0:128])
                    nc.vector.tensor_copy(bcs[:, 128:512], bps[:, 128:512])
                    nc.vector.tensor_mul(ctxT[0:64, q0 + 128:q0 + 512],
                                         ctxA[0:64, 128:512],
                                         bcs[0:64, 128:512])
                    nc.vector.tensor_mul(ctxT[64:128, q0 + 128:q0 + 512],
                                         ctxB[64:128, 128:512],
                                         bcs[64:128, 128:512])
                return norm_tail

            # ---- main schedule ----
            for _, u in qkv_units(0, paced=True):
                u()
            # proj filler assignment per block index 0..7 (block = 4b+qj):
            # every block hosts the previous block's projection; the
            # ACT-bound nk=16 block 7 has PE slack for two blocks' proj.
            # gate=2 delays pops until the producing norm (run at ki==1 via
            # `pending`) is in the queue.
            proj_fill = {1: [((0, 0), 3, True)], 2: [((0, 1), 3, True)],
                         3: [((0, 2), 3, False)], 4: [((0, 3), 3, True)],
                         5: [((1, 0), 3, True)],
                         7: [((1, 1), 0, False), ((1, 2), 3, False)]}
            def mk_la(nb, nqj):
                def la():
                    emit_score_exp(nb, nqj, 0)
                    emit_score_exp(nb, nqj, 1)
                return la

            pending = None
            v7_units = None
            for b in range(B):
                for qj in range(4):
                    tt = 4 * b + qj
                    if tt + 2 <= 7:
                        prefetch(tt + 2)
                    fq = deque()
                    if tt == 6:
                        # tile 7's v/vnat units become block 7's PE filler
                        # (block 7 otherwise starves and HAM-throttles);
                        # its vnat isn't read before ki=12 there
                        qk7, v7_units = qkv_units(7, split_v=True)
                        fq.extend(qk7)
                    elif tt + 1 <= 7:
                        fq.extend(qkv_units(tt + 1))
                    if tt == 7:
                        fq.extend(v7_units)
                    for (pb, pq), gate, ae in proj_fill.get(tt, []):
                        fq.extend(proj_units(pb, pq, gate, ae))
                    la = (mk_la((tt + 1) // 4, (tt + 1) % 4)
                          if tt < 7 else None)
                    pending = attention(b, qj, fq, pending, la)

            # ---- tail: norm + proj of block (1,3) with PE keepalives so
            # the HAM clock stays at 8/8 through the serial ACT/DVE chain
            kps = scp_pool.tile([128, 512], F32, tag="sc", name="keep")

            def keep(n):
                for _ in range(n):
                    nc.tensor.matmul(kps[0:128, 0:128], lhsT=junk, rhs=junk,
                                     start=True, stop=True)

            keep(22)
            pending()
            keep(8)
            # final projection: alternate psum pools (fillp/ctxp) so the 8
            # matmuls pipeline across 4 live buffers instead of stalling on
            # the 2-buffer fill pool behind each chunk's evac
            for t in range(4):
                tc_ = 28 + t
                ot = outp.tile([128, 1024], BF, tag="ot", name=f"ot{tc_}")
                for n2 in range(2):
                    if n2 == 0:
                        pp = fillp.tile([128, 512], F32, tag="f",
                                        name=f"pj{tc_}_0")
                    else:
                        pp = ctxp.tile([128, 512], F32, tag="ctx",
                                       name=f"pj{tc_}_1")
                    nc.tensor.matmul(
                        pp, lhsT=ctxT[:, tc_ * 128:(tc_ + 1) * 128],
                        rhs=wpT_sb[:, n2 * 512:(n2 + 1) * 512],
                        start=True, stop=True,
                    )
                    if n2 == 0:
                        nc.vector.tensor_copy(ot[:, 0:512], pp)
                    else:
                        nc.scalar.activation(out=ot[:, 512:1024], in_=pp,
                                             func=CPY)
                        nc.sync.dma_start(
                            out=out[tc_ * 128:(tc_ + 1) * 128, :], in_=ot)
                keep(2)
            keep(8)
    nc.finalize()
    _built["nc"] = nc
    return nc


def kernel(hidden_states, attention_mask, W_attn, b_attn, W_proj, b_proj,
           _trace=False):
    hs = np.asarray(hidden_states, np.float32).reshape(T, E)
    # [tile, partition, kc, col] with 8KB contiguous per partition line
    hsTt = np.ascontiguousarray(
        hs.reshape(8, 512, 8, 128).transpose(0, 3, 2, 1).reshape(8, 128, 4096)
    ).astype(BF16)
    mask = np.asarray(attention_mask)
    mcol = (mask.reshape(B * S) != 0).astype(np.float32)        # [4096]
    mchunk = np.ascontiguousarray(mcol.reshape(32, 128).T)       # [128, 32]
    maskb = mchunk.astype(BF16)
    masks_ = mchunk.astype(np.float32)
    W_attn = np.asarray(W_attn, np.float32)
    W_proj = np.asarray(W_proj, np.float32)
    b_attn = np.asarray(b_attn, np.float32)

    in_maps = []
    for c in range(NCORE):
        rows = np.concatenate(
            [np.arange(sec * E + c * 128, sec * E + (c + 1) * 128)
             for sec in range(3)]
        )
        wq = np.ascontiguousarray(
            W_attn[rows].T.reshape(8, 128, 384).transpose(1, 0, 2)
        ).astype(BF16)                                               # [128,8,384]
        bq = np.ascontiguousarray(
            b_attn[rows].reshape(3, 128).T).astype(np.float32)             # [128,3] f32
        wp = np.ascontiguousarray(W_proj[:, c * 128:(c + 1) * 128].T).astype(BF16)
        in_maps.append(
            {"hsTt": hsTt, "wqkv": wq, "bqkv": bq, "wpT": wp,
             "maskb": maskb, "masks": masks_}
        )

    nc = _build()
    res = run_bass_kernel_spmd(nc, in_maps, list(range(NCORE)), trace=_trace)
    parts = np.stack([np.asarray(r["out"], np.float32) for r in res.results])
    outv = parts.sum(axis=0) + np.asarray(b_proj, np.float32)[None, :]
    out = outv.reshape(B, S, E).astype(np.float32)
    if _trace:
        return out, res
    return out


# revision 41
# speedup vs baseline: 1.1858x; 1.0192x over previous
"""GPT2 attention (B=2, S=2048, E=1024, H=16) on 8 NeuronCores.

Sharding: tensor-parallel over heads - 2 heads per core. Each core computes
qkv^T for its heads, causal attention in transposed-score layout (k on
partitions, q on free dim), then a partial output projection over its 128
ctx dims. Host sums the 8 partials and adds b_proj.

v3 design notes (vs the 183us v2):
- ACT (scalar engine) decongested: it now runs ONLY the 80 exps. The proj
  psum evacuations, softmax-denominator copy and bf16 recip cast moved to
  DVE. The nk=16 blocks were ACT-bound end to end (16 exps + 4-8 evac
  copies ~= the whole block span); now they are bound by exp alone.
- Filler rebalance: every attention block hosts one qkv tile and/or one
  block's output projection (previously block (1,1) had no filler at all
  and stalled the PE long enough for the HAM clock to halve). proj filler
  units carry a gate_ki so they can't pop before the producing norm ran.
- Warmup no longer waits for make_identity: dummy matmuls read a junk
  tile, so the PE ramps the HAM clock from the very start of the body
  while the first DMAs are still in flight.
- DMA order: wqkv first half, first hsT quarters, then biases/masks, so
  the first QKV matmul has its operands ~3us earlier.
- Tail: keepalive matmuls interleaved into the final norm+proj chain keep
  the HAM clock at full rate while ACT/DVE finish up.
- PV matmuls carry the denominator in-psum (mask column in the vnat
  stationaries); scores run as row-tiled concurrent h0/h1 pairs.
"""
import os
from collections import deque

import numpy as np
import ml_dtypes

import concourse.bass as bass
import concourse.bacc as bacc
import concourse.tile as tile
from concourse import mybir
from concourse import masks
from concourse.bass_utils import run_bass_kernel_spmd

BF16 = ml_dtypes.bfloat16
B, S, E, H, D = 2, 2048, 1024, 16, 64
T = B * S                 # 4096 tokens
NCORE = 8
HPC = H // NCORE          # 2 heads per core
SCALE = D ** -0.5
F32 = mybir.dt.float32
BF = mybir.dt.bfloat16
EXP = mybir.ActivationFunctionType.Exp
CPY = mybir.ActivationFunctionType.Copy
N_WARMUP = 45

_built = {}


def _build():
    if "nc" in _built:
        return _built["nc"]
    nc = bacc.Bacc()
    hsTt = nc.declare_dram_parameter("hsTt", [8, 128, 8 * 512], BF,
                                     isOutput=False)
    wqkv = nc.declare_dram_parameter("wqkv", [128, 8, 3 * HPC * D], BF,
                                     isOutput=False)
    bqkv = nc.declare_dram_parameter("bqkv", [128, 3], F32, isOutput=False)
    wpT = nc.declare_dram_parameter("wpT", [HPC * D, E], BF, isOutput=False)
    maskb = nc.declare_dram_parameter("maskb", [128, 32], BF, isOutput=False)
    masks_ = nc.declare_dram_parameter("masks", [128, 32], F32, isOutput=False)
    out = nc.declare_dram_parameter("out", [T, E], BF, isOutput=True)


    with tile.TileContext(nc) as tc:
        with (
            tc.tile_pool(name="const", bufs=1) as constp,
            tc.tile_pool(name="big", bufs=1) as bigp,
            tc.tile_pool(name="hst", bufs=4) as hstp,
            tc.tile_pool(name="vtmp", bufs=2) as vtmpp,
            tc.tile_pool(name="expt", bufs=5) as exptp,
            tc.tile_pool(name="outp", bufs=4) as outp,
            tc.tile_pool(name="bcs", bufs=2) as bcsp,
            tc.tile_pool(name="recf", bufs=2) as recfp,
            tc.tile_pool(name="recb", bufs=2) as recbp,
            tc.tile_pool(name="ps_fill", bufs=2, space="PSUM") as fillp,
            tc.tile_pool(name="ps_sc", bufs=2, space="PSUM") as scp_pool,
            tc.tile_pool(name="ps_ctx", bufs=2, space="PSUM") as ctxp,
        ):
            # ---- constants ----
            wqkv_sb = constp.tile([128, 8, 384], BF)
            bq_sb = constp.tile([128, 3], F32)
            wpT_sb = constp.tile([128, E], BF)
            msk_sb = constp.tile([128, 32], F32)
            mskb_sb = constp.tile([128, 32], BF)
            ident = constp.tile([128, 128], BF)
            ones_sb = constp.tile([128, 64], BF)
            junk = constp.tile([128, 128], BF)

            qT = bigp.tile([128, T], BF)       # rows: h0 dims 0-63, h1 dims 64-127
            kT = bigp.tile([128, T], BF)
            ctxT = bigp.tile([128, T], BF)
            # vnatA: per 128-token chunk: [h0 dims 0:64, mask col 64]
            # vnatB: per 128-token chunk: [mask col 0, zeros 1:64, h1 dims 64:128]
            vnatA = bigp.tile([128, 32, 65], BF)
            vnatB = bigp.tile([128, 32, 128], BF)
            den2 = [bigp.tile([128, 512], F32, name=f"den{i}") for i in range(2)]

            # ---- hsT tile prefetch ----
            hst_tiles = {}

            def prefetch(n, quarters=False):
                ht = hstp.tile([128, 8, 512], BF, tag="ht", name=f"ht{n}")
                src_v = hsTt[n].rearrange("p (kc t) -> p kc t", kc=8)
                if quarters:
                    for q in range(4):
                        nc.sync.dma_start(out=ht[:, 2 * q:2 * q + 2, :],
                                          in_=src_v[:, 2 * q:2 * q + 2, :])
                else:
                    nc.sync.dma_start(out=ht[:, 0:4, :], in_=src_v[:, 0:4, :])
                    nc.sync.dma_start(out=ht[:, 4:8, :], in_=src_v[:, 4:8, :])
                hst_tiles[n] = ht

            # PE pre-warm on a junk tile (no dependency on DMA or identity
            # build): ramps the HAM clock gate to 8/8 while data streams in
            nc.vector.memset(junk, 1.0)
            wps = fillp.tile([128, 128], F32, tag="f", name="warm")

            def keep0(n):
                for _ in range(n):
                    nc.tensor.matmul(wps, lhsT=junk, rhs=junk,
                                     start=True, stop=True)

            keep0(N_WARMUP)

            # DMA order: k-major interleave of wqkv / hsT-tile-0 quarters so
            # the first QKV matmuls start as soon as ~350KB have landed and
            # then pace with the DMA stream
            ht0 = hstp.tile([128, 8, 512], BF, tag="ht", name="ht0")
            src0 = hsTt[0].rearrange("p (kc t) -> p kc t", kc=8)
            for q in range(4):
                nc.sync.dma_start(out=wqkv_sb[:, 2 * q:2 * q + 2, :],
                                  in_=wqkv[:, 2 * q:2 * q + 2, :])
                nc.sync.dma_start(out=ht0[:, 2 * q:2 * q + 2, :],
                                  in_=src0[:, 2 * q:2 * q + 2, :])
            hst_tiles[0] = ht0
            nc.sync.dma_start(out=bq_sb, in_=bqkv[:])
            nc.sync.dma_start(out=msk_sb, in_=masks_[:])
            nc.sync.dma_start(out=mskb_sb, in_=maskb[:])
            prefetch(1)
            nc.sync.dma_start(out=wpT_sb, in_=wpT[:])
            masks.make_identity(nc, ident[:])
            nc.vector.memset(ones_sb, 1.0)
            nc.vector.memset(den2[0], 1.0)
            nc.vector.memset(den2[1], 1.0)
            nc.gpsimd.memset(vnatB[:, :, 1:64], 0.0)
            nc.vector.tensor_copy(vnatA[:, :, 64:65], mskb_sb)
            nc.vector.tensor_copy(vnatB[:, :, 0:1], mskb_sb)

            # ---- qkv tile for 512 tokens: filler units ----
            vtmp_of = {}

            def qkv_units(n, paced=False, split_v=False):
                units = []
                pm = {}

                def mk_mm(m, klo, khi):
                    def u():
                        if klo == 0:
                            pm[m] = fillp.tile([128, 512], F32, tag="f",
                                               name=f"qkv{n}_{m}")
                        for k in range(klo, khi):
                            nc.tensor.matmul(
                                pm[m], lhsT=wqkv_sb[:, k, m * 128:(m + 1) * 128],
                                rhs=hst_tiles[n][:, k, :],
                                start=(k == 0), stop=(k == 7),
                            )
                    return u

                def mk_ev(m):
                    def u():
                        if m == 0:
                            nc.vector.tensor_scalar_add(
                                qT[:, n * 512:(n + 1) * 512], pm[0], bq_sb[:, 0:1])
                        elif m == 1:
                            nc.vector.tensor_scalar_add(
                                kT[:, n * 512:(n + 1) * 512], pm[1], bq_sb[:, 1:2])
                        else:
                            vt = vtmpp.tile([128, 512], BF, tag="vt",
                                            name=f"vt{n}")
                            nc.vector.tensor_scalar_add(vt, pm[2], bq_sb[:, 2:3])
                            vtmp_of[n] = vt
                    return u

                def mk_tr(t):
                    def u():
                        pst = fillp.tile([128, 128], BF, tag="f",
                                         name=f"tr{n}_{t}")
                        nc.tensor.transpose(
                            pst[:], vtmp_of[n][:, t * 128:(t + 1) * 128], ident[:])
                        tt4 = n * 4 + t
                        nc.vector.tensor_scalar_mul(
                            vnatA[:, tt4, 0:64], pst[:, 0:64],
                            msk_sb[:, tt4:tt4 + 1])
                        nc.vector.tensor_scalar_mul(
                            vnatB[:, tt4, 64:128], pst[:, 64:128],
                            msk_sb[:, tt4:tt4 + 1])
                    return u

                for m in range(3):
                    for klo in range(0, 8, 2):
                        units.append((0, mk_mm(m, klo, klo + 2)))
                        if paced and m == 0 and klo < 6:
                            # cover the DMA-arrival gaps of the k-major
                            # quarter stream with PE keepalives
                            units.append((0, lambda: keep0(6)))
                    units.append((0, mk_ev(m)))
                for t in range(4):
                    units.append((0, mk_tr(t)))
                if split_v:
                    # [qk-part, v-part]: the v/vnat units can run a block
                    # later than q/k (vnat is only read by the PVs)
                    return units[:10], units[10:]
                return units

            # ---- output projection for one qj block: filler units ----
            ot_of = {}

            def proj_units(b, qj, gate=0, act_evac=True):
                # act_evac=False keeps the n2=1 evac on DVE: used when the
                # hosting block is ACT-bound (nk=16), so ACT only runs exps
                units = []

                def mk_pj(t, n2):
                    tc_ = (4 * b + qj) * 4 + t

                    def u():
                        pp = fillp.tile([128, 512], F32, tag="f",
                                        name=f"pj{tc_}_{n2}")
                        nc.tensor.matmul(
                            pp, lhsT=ctxT[:, tc_ * 128:(tc_ + 1) * 128],
                            rhs=wpT_sb[:, n2 * 512:(n2 + 1) * 512],
                            start=True, stop=True,
                        )
                        if n2 == 0:
                            ot = outp.tile([128, 1024], BF, tag="ot",
                                           name=f"ot{tc_}")
                            ot_of[tc_] = ot
                            nc.vector.tensor_copy(ot[:, 0:512], pp)
                        else:
                            ot = ot_of.pop(tc_)
                            if act_evac:
                                nc.scalar.activation(out=ot[:, 512:1024],
                                                     in_=pp, func=CPY)
                            else:
                                nc.vector.tensor_copy(ot[:, 512:1024], pp)
                            nc.sync.dma_start(
                                out=out[tc_ * 128:(tc_ + 1) * 128, :],
                                in_=ot,
                            )
                    return u

                for t in range(4):
                    for n2 in range(2):
                        units.append((gate, mk_pj(t, n2)))
                return units

            # ---- score+exp chain for chunk ki of block (b, qj) ----
            # shared by the in-loop path and the cross-block lookahead
            exps_g = {}

            def emit_score_exp(b, qj, ki):
                scp = scp_pool.tile([128, 1024], F32, tag="sc",
                                    name=f"sc{b}{qj}_{ki}")
                qsl = slice(b * S + qj * 512, b * S + (qj + 1) * 512)
                ksl = slice(b * S + ki * 128, b * S + (ki + 1) * 128)
                nc.tensor.matmul(scp[:, 0:512], lhsT=kT[0:64, ksl],
                                 rhs=qT[0:64, qsl], start=True, stop=True)
                nc.tensor.matmul(scp[:, 512:1024], lhsT=kT[64:128, ksl],
                                 rhs=qT[64:128, qsl], start=True, stop=True)
                e = exptp.tile([128, 1024], BF, tag="e",
                               name=f"ex{b}{qj}_{ki}")
                nc.scalar.activation(out=e, in_=scp, func=EXP, scale=SCALE)
                d = ki - 4 * qj
                if d >= 0:   # diagonal: zero where k > q
                    for hh in range(2):
                        sl = e[:, hh * 512:(hh + 1) * 512]
                        nc.gpsimd.affine_select(
                            out=sl, in_=sl,
                            compare_op=mybir.AluOpType.is_ge, fill=0.0,
                            base=-(128 * d), channel_multiplier=-1,
                            pattern=[[1, 512]],
                        )
                exps_g[(b, qj, ki)] = e

            # ---- causal attention for one (b, qj) 512-query block ----
            # la: closure emitting the NEXT block's first score+exp chains
            # before this block's PV tail, so the next block's ACT work
            # starts ~2us earlier (the boundary was an ACT bubble).
            def attention(b, qj, fq, pending=None, la=None, final=False):
                nk = 4 * qj + 4
                ctxA = ctxp.tile([128, 512], F32, tag="ctx", name=f"cA{b}{qj}")
                ctxB = ctxp.tile([128, 512], F32, tag="ctx", name=f"cB{b}{qj}")
                qsl = slice(b * S + qj * 512, b * S + (qj + 1) * 512)

                def pop_filler(k, ki):
                    for _ in range(k):
                        if fq and fq[0][0] <= ki:
                            fq.popleft()[1]()

                def emit_pv(ki):
                    e = exps_g.pop((b, qj, ki))
                    kc = b * 16 + ki
                    nc.tensor.matmul(
                        ctxA[0:65, :], lhsT=vnatA[:, kc, :], rhs=e[:, 0:512],
                        start=(ki == 0), stop=(ki == nk - 1),
                    )
                    nc.tensor.matmul(
                        ctxB[:, :], lhsT=vnatB[:, kc, :], rhs=e[:, 512:1024],
                        start=(ki == 0), stop=(ki == nk - 1),
                    )

                for ki in range(nk):
                    if (b, qj, ki) not in exps_g:
                        emit_score_exp(b, qj, ki)
                    if ki == 1 and pending is not None:
                        pending()
                    slots_left = nk - ki + 1
                    pop_filler(min(4, -(-len(fq) // slots_left)), ki)
                    if ki >= 2:
                        emit_pv(ki - 2)
                # lookahead: next block's ki=0,1 score+exp before our tail
                if la is not None:
                    la()
                emit_pv(nk - 2)
                pop_filler(max(0, len(fq) - 4), nk)
                emit_pv(nk - 1)
                # drain most filler BEFORE the norm chain below so the next
                # block's qT/kT evacs aren't queued behind it on DVE (this
                # was a ~2us PE stall at every block boundary). For the
                # FINAL block there is no next block: the norm chain goes
                # first on DVE (recb ~2.6us earlier) and the leftover proj
                # matmuls become organic PE keepalives behind it.
                if not final:
                    pop_filler(max(0, len(fq) - 4), nk)

                # normalize part 1 (no PE ops): h0 denom = ctxA row 64,
                # h1 denom = ctxB row 0. ACT copies row 64 / DVE row 0 (rows
                # 64/0 of a memset-once tile), one base-0 approx-reciprocal
                # covers both rows, one ACT cast to bf16 for the broadcast.
                # split ACT/DVE: at block ends DVE is congested (filler
                # evacs, vnat muls) while ACT only has the lookahead exps
                den = den2[(4 * b + qj) % 2]
                nc.scalar.activation(out=den[64:65, :], in_=ctxA[64:65, :],
                                     func=CPY)
                nc.vector.tensor_copy(den[0:1, :], ctxB[0:1, :])
                recf = recfp.tile([128, 512], F32, tag="rf", name=f"rf{b}{qj}")
                recb = recbp.tile([128, 512], BF, tag="rb", name=f"rb{b}{qj}")
                nc.vector.reciprocal_approx_fast(recf[0:65, :], den[0:65, :])
                with nc.allow_low_precision(reason="bf16 recip, rel<2e-2 ok"):
                    nc.scalar.activation(out=recb[0:65, :], in_=recf[0:65, :],
                                         func=CPY)
                pop_filler(len(fq), nk)

                def norm_tail():
                    # part 2 (PE bcast + DVE muls) - deferred into the next
                    # attention block so the PE queue never stalls on recips
                    bps = scp_pool.tile([128, 512], F32, tag="sc",
                                        name=f"bp{b}{qj}")
                    nc.tensor.matmul(bps[0:64, :], lhsT=ones_sb[64:65, :],
                                     rhs=recb[64:65, :], start=True, stop=True)
                    nc.tensor.matmul(bps[64:128, :], lhsT=ones_sb[0:1, :],
                                     rhs=recb[0:1, :], start=True, stop=True)
                    bcs = bcsp.tile([128, 512], BF, tag="bc", name=f"bc{b}{qj}")
                    # split the broadcast-evac and the muls so the first
                    # 128-token chunk's ctxT (what the first proj matmul
                    # reads) is ready ~1.2us earlier
                    q0 = qsl.start
                    nc.vector.tensor_copy(bcs[:, 0:128], bps[:, 0:128])
                    nc.vector.tensor_mul(ctxT[0:64, q0:q0 + 128],
                                         ctxA[0:64, 0:128], bcs[0:64, 0:128])
                    nc.vector.tensor_mul(ctxT[64:128, q0:q0 + 128],
                                         ctxB[64:128, 0:128],
                                         bcs[64:128, 0:128])
                    nc.vector.tensor_copy(bcs[:, 128:512], bps[:, 128:512])
                    nc.vector.tensor_mul(ctxT[0:64, q0 + 128:q0 + 512],
                                         ctxA[0:64, 128:512],
                                         bcs[0:64, 128:512])
                    nc.vector.tensor_mul(ctxT[64:128, q0 + 128:q0 + 512],
                                         ctxB[64:128, 128:512],
                                         bcs[64:128, 128:512])
                return norm_tail

            # ---- main schedule ----
            for _, u in qkv_units(0, paced=True):
                u()
            # proj filler assignment per block index 0..7 (block = 4b+qj):
            # every block hosts the previous block's projection; the
            # ACT-bound nk=16 block 7 has PE slack for two blocks' proj.
            # gate=2 delays pops until the producing norm (run at ki==1 via
            # `pending`) is in the queue.
            proj_fill = {1: [((0, 0), 3, True)], 2: [((0, 1), 3, True)],
                         3: [((0, 2), 3, False)], 4: [((0, 3), 3, True)],
                         5: [((1, 0), 3, True)],
                         7: [((1, 1), 0, False), ((1, 2), 3, False)]}
            def mk_la(nb, nqj):
                def la():
                    emit_score_exp(nb, nqj, 0)
                    emit_score_exp(nb, nqj, 1)
                return la

            pending = None
            v7_units = None
            for b in range(B):
                for qj in range(4):
                    tt = 4 * b + qj
                    if tt + 2 <= 7:
                        prefetch(tt + 2)
                    fq = deque()
                    if tt == 6:
                        # tile 7's v/vnat units become block 7's PE filler
                        # (block 7 otherwise starves and HAM-throttles);
                        # its vnat isn't read before ki=12 there
                        qk7, v7_units = qkv_units(7, split_v=True)
                        fq.extend(qk7)
                    elif tt + 1 <= 7:
                        fq.extend(qkv_units(tt + 1))
                    if tt == 7:
                        fq.extend(v7_units)
                    for (pb, pq), gate, ae in proj_fill.get(tt, []):
                        fq.extend(proj_units(pb, pq, gate, ae))
                    la = (mk_la((tt + 1) // 4, (tt + 1) % 4)
                          if tt < 7 else None)
                    pending = attention(b, qj, fq, pending, la,
                                        final=(tt == 7))

            # ---- tail: norm + proj of block (1,3) with PE keepalives so
            # the HAM clock stays at 8/8 through the serial ACT/DVE chain
            kps = scp_pool.tile([128, 512], F32, tag="sc", name="keep")

            def keep(n):
                for _ in range(n):
                    nc.tensor.matmul(kps[0:128, 0:128], lhsT=junk, rhs=junk,
                                     start=True, stop=True)

            keep(6)
            pending()
            keep(8)
            # final projection: alternate psum pools (fillp/ctxp) so the 8
            # matmuls pipeline across 4 live buffers instead of stalling on
            # the 2-buffer fill pool behind each chunk's evac
            for t in range(4):
                tc_ = 28 + t
                ot = outp.tile([128, 1024], BF, tag="ot", name=f"ot{tc_}")
                for n2 in range(2):
                    if n2 == 0:
                        pp = fillp.tile([128, 512], F32, tag="f",
                                        name=f"pj{tc_}_0")
                    else:
                        pp = ctxp.tile([128, 512], F32, tag="ctx",
                                       name=f"pj{tc_}_1")
                    nc.tensor.matmul(
                        pp, lhsT=ctxT[:, tc_ * 128:(tc_ + 1) * 128],
                        rhs=wpT_sb[:, n2 * 512:(n2 + 1) * 512],
                        start=True, stop=True,
                    )
                    if n2 == 0:
                        nc.vector.tensor_copy(ot[:, 0:512], pp)
                    else:
                        nc.scalar.activation(out=ot[:, 512:1024], in_=pp,
                                             func=CPY)
                        nc.sync.dma_start(
                            out=out[tc_ * 128:(tc_ + 1) * 128, :], in_=ot)
                keep(2)
            keep(8)
    nc.finalize()
    _built["nc"] = nc
    return nc


def kernel(hidden_states, attention_mask, W_attn, b_attn, W_proj, b_proj,
           _trace=False):
    hs = np.asarray(hidden_states, np.float32).reshape(T, E)
    # [tile, partition, kc, col] with 8KB contiguous per partition line
    hsTt = np.ascontiguousarray(
        hs.reshape(8, 512, 8, 128).transpose(0, 3, 2, 1).reshape(8, 128, 4096)
    ).astype(BF16)
    mask = np.asarray(attention_mask)
    mcol = (mask.reshape(B * S) != 0).astype(np.float32)        # [4096]
    mchunk = np.ascontiguousarray(mcol.reshape(32, 128).T)       # [128, 32]
    maskb = mchunk.astype(BF16)
    masks_ = mchunk.astype(np.float32)
    W_attn = np.asarray(W_attn, np.float32)
    W_proj = np.asarray(W_proj, np.float32)
    b_attn = np.asarray(b_attn, np.float32)

    in_maps = []
    for c in range(NCORE):
        rows = np.concatenate(
            [np.arange(sec * E + c * 128, sec * E + (c + 1) * 128)
             for sec in range(3)]
        )
        wq = np.ascontiguousarray(
            W_attn[rows].T.reshape(8, 128, 384).transpose(1, 0, 2)
        ).astype(BF16)                                               # [128,8,384]
        bq = np.ascontiguousarray(
            b_attn[rows].reshape(3, 128).T).astype(np.float32)             # [128,3] f32
        wp = np.ascontiguousarray(W_proj[:, c * 128:(c + 1) * 128].T).astype(BF16)
        in_maps.append(
            {"hsTt": hsTt, "wqkv": wq, "bqkv": bq, "wpT": wp,
             "maskb": maskb, "masks": masks_}
        )

    nc = _build()
    res = run_bass_kernel_spmd(nc, in_maps, list(range(NCORE)), trace=_trace)
    parts = np.stack([np.asarray(r["out"], np.float32) for r in res.results])
    outv = parts.sum(axis=0) + np.asarray(b_proj, np.float32)[None, :]
    out = outv.reshape(B, S, E).astype(np.float32)
    if _trace:
        return out, res
    return out
